# revision 13
# baseline (speedup 1.0000x reference)
"""BiLSTM-CRF Trainium2 kernel.

Sharding: 8 cores = 2 directions x 4 sequence-groups (8 seqs each).
Core c in 0..3: forward LSTM for seqs [8c, 8c+8); core c+4: backward
LSTM for the same group (host feeds it time-reversed tokens).
Each core: embedding gather -> input projection -> LSTM recurrence
(weight-stationary fp32 matmuls) -> partial CRF emission features;
pair AllReduce combines fwd+bwd partials; every core then runs the
Viterbi scan + backtrace for its group's 8 sequences. Host takes
cores 0-3's outputs.
"""

import os
import numpy as np

import concourse.bacc as bacc
import concourse.bass as bass
import concourse.mybir as mybir
import concourse.tile as tile
from concourse.bass_utils import run_bass_kernel_spmd
from concourse.masks import make_identity

f32 = mybir.dt.float32
i32 = mybir.dt.int32
Alu = mybir.AluOpType
Act = mybir.ActivationFunctionType
AX = mybir.AxisListType

B, S = 32, 256
EMB = 256
H = 256          # per-direction hidden
G4 = 4 * H       # 1024 gates
TAGS = 12
START_IDX, STOP_IDX = 10, 11
NEG = -10000.0
BIGF = 1.0e9

NCORES = 8
GRP = 4          # seq groups
BS = B // GRP    # 8 seqs per core
TOK = BS * S     # 2048 tokens per core
NT = TOK // 128  # 16 token tiles

NSTEPS = int(os.environ.get("KERNEL_NSTEPS", str(S)))

_cache = {}


def _bcast_ap(ap, nrep):
    """Replicate a [1, ...]-partition dram AP across nrep partitions."""
    return bass.AP(tensor=ap.tensor, offset=ap.offset, ap=[[0, nrep]] + list(ap.ap))


def _free_bcast(t_ap, reps, inner):
    """AP reading t_ap ([P, inner]) as [P, reps, inner] with 0-stride reps."""
    return bass.AP(
        tensor=t_ap.tensor,
        offset=t_ap.offset,
        ap=[list(t_ap.ap[0]), [0, reps], [1, inner]],
    )


def _inner_bcast(t_ap, outer, reps):
    """AP reading t_ap ([P, outer]) as [P, outer, reps] with 0-stride reps."""
    return bass.AP(
        tensor=t_ap.tensor,
        offset=t_ap.offset,
        ap=[list(t_ap.ap[0]), [1, outer], [0, reps]],
    )


def _build():
    nc = bacc.Bacc(None, target_bir_lowering=False, enable_partition_id=False)

    emb_d = nc.dram_tensor("emb_tab", [100000, EMB], f32, kind="ExternalInput")
    idx_d = nc.dram_tensor("sent_idx", [TOK], i32, kind="ExternalInput")
    scat_d = nc.dram_tensor("scat_idx", [TOK], i32, kind="ExternalInput")
    wihT_d = nc.dram_tensor("wihT", [EMB, G4], f32, kind="ExternalInput")
    whhT_d = nc.dram_tensor("whhT", [H, G4], f32, kind="ExternalInput")
    bias_d = nc.dram_tensor("bias", [8, 128], f32, kind="ExternalInput")
    woT_d = nc.dram_tensor("woT", [H, TAGS], f32, kind="ExternalInput")
    bo_d = nc.dram_tensor("bo", [TAGS], f32, kind="ExternalInput")
    trans_d = nc.dram_tensor("trans_flat", [TAGS * TAGS], f32, kind="ExternalInput")
    tstop_d = nc.dram_tensor("trans_stop", [TAGS], f32, kind="ExternalInput")
    fvinit_d = nc.dram_tensor("fv_init", [TAGS], f32, kind="ExternalInput")
    iota144_d = nc.dram_tensor("iota144", [TAGS * TAGS], f32, kind="ExternalInput")
    iota12_d = nc.dram_tensor("iota12", [TAGS], f32, kind="ExternalInput")

    pfz_d = nc.dram_tensor("pf_zero", [GRP * TOK, TAGS], f32, kind="ExternalInput")

    score_d = nc.dram_tensor("score", [B, 1], f32, kind="ExternalOutput")
    path_d = nc.dram_tensor("path", [B, S], i32, kind="ExternalOutput")

    pf_d = nc.dram_tensor("pf_buf", [GRP * TOK, TAGS], f32)
    feats_d = nc.dram_tensor("feats_buf", [GRP * TOK, TAGS], f32, addr_space="Shared")

    with tile.TileContext(nc) as tc:
        consts_cm = tc.tile_pool(name="consts", bufs=1)
        consts = consts_cm.__enter__()
        wihT = consts.tile([128, 2, G4], f32)
        nc.gpsimd.dma_start(wihT[:], wihT_d.rearrange("(a b) g -> b a g", b=128))
        whhT = consts.tile([128, 2, G4], f32)
        nc.gpsimd.dma_start(whhT[:], whhT_d.rearrange("(a b) g -> b a g", b=128))
        bias = consts.tile([128, 8], f32)
        nc.gpsimd.dma_start(bias[:], bias_d.rearrange("m p -> p m"))
        woT = consts.tile([128, 2, TAGS], f32)
        nc.gpsimd.dma_start(woT[:], woT_d.rearrange("(a b) n -> b a n", b=128))
        bo_rep = consts.tile([128, TAGS], f32)
        nc.gpsimd.dma_start(bo_rep[:], _bcast_ap(bo_d[:], 128))
        idx_sb = consts.tile([128, NT], i32)
        nc.gpsimd.dma_start(idx_sb[:], idx_d.rearrange("(g p) -> p g", p=128))
        scat_sb = consts.tile([128, NT], i32)
        nc.gpsimd.dma_start(scat_sb[:], scat_d.rearrange("(g p) -> p g", p=128))
        ident = consts.tile([128, 128], f32)
        make_identity(nc, ident[:])

        trans_rep = consts.tile([32, TAGS * TAGS], f32)
        nc.gpsimd.dma_start(trans_rep[:], _bcast_ap(trans_d[:], 32))
        iota144 = consts.tile([32, TAGS * TAGS], f32)
        nc.gpsimd.dma_start(iota144[:], _bcast_ap(iota144_d[:], 32))
        iota12 = consts.tile([32, TAGS], f32)
        nc.gpsimd.dma_start(iota12[:], _bcast_ap(iota12_d[:], 32))
        tstop_rep = consts.tile([32, TAGS], f32)
        nc.gpsimd.dma_start(tstop_rep[:], _bcast_ap(tstop_d[:], 32))
        big_rep = consts.tile([32, TAGS * TAGS], f32)
        nc.vector.memset(big_rep[:], BIGF)

        # big persistent state
        xg = consts.tile([128, 8, TOK], f32)          # pre-biased input projection
        hs = consts.tile([128, 2, BS * (S + 1)], f32)  # h_{-1..S-1}, slot(t)=8(t+1)
        c_sb = consts.tile([128, 2, BS], f32)
        nc.vector.memset(hs[:], 0.0)
        nc.vector.memset(c_sb[:], 0.0)

        # ---- P0/P1: embedding gather + transpose ----
        with (
            tc.tile_pool(name="emb_pool", bufs=3) as embp,
            tc.tile_pool(name="tp_psum", bufs=2, space="PSUM") as tpp,
        ):
            xT = consts.tile([128, 2, TOK], f32)
            for g in range(NT):
                xrow = embp.tile([128, EMB], f32, name=f"xrow{g}", tag="xrow")
                nc.gpsimd.indirect_dma_start(
                    out=xrow[:],
                    out_offset=None,
                    in_=emb_d[:],
                    in_offset=bass.IndirectOffsetOnAxis(ap=idx_sb[:, g : g + 1], axis=0),
                )
                for kc in range(2):
                    pt = tpp.tile([128, 128], f32, name=f"pt{g}_{kc}", tag=f"pt{kc}")
                    nc.tensor.transpose(
                        out=pt[:], in_=xrow[:, 128 * kc : 128 * (kc + 1)], identity=ident[:]
                    )
                    nc.vector.tensor_copy(xT[:, kc, 128 * g : 128 * (g + 1)], pt[:])

        # ---- P2: input projection xg = wih @ x + bias ----
        with tc.tile_pool(name="xg_psum", bufs=2, space="PSUM") as xgp:
            for m in range(8):
                for nb in range(NT):
                    px = xgp.tile([128, 128], f32, name=f"px{m}_{nb}", tag=f"px{nb % 4}")
                    for kc in range(2):
                        nc.tensor.matmul(
                            px[:],
                            wihT[:, kc, 128 * m : 128 * (m + 1)],
                            xT[:, kc, 128 * nb : 128 * (nb + 1)],
                            start=(kc == 0),
                            stop=(kc == 1),
                        )
                    nc.vector.tensor_scalar(
                        out=xg[:, m, 128 * nb : 128 * (nb + 1)],
                        in0=px[:],
                        scalar1=bias[:, m : m + 1],
                        scalar2=None,
                        op0=Alu.add,
                    )

        # ---- P3: LSTM recurrence ----
        with (
            tc.tile_pool(name="rec_psum", bufs=1, space="PSUM") as rpp,
            tc.tile_pool(name="rec_sbuf", bufs=3) as rsb,
        ):
            accs = [rpp.tile([128, BS], f32, name=f"racc{m}", tag=f"racc{m}") for m in range(8)]
            WAVES = ((0, 2, 4, 6), (1, 3, 5, 7))
            for t in range(NSTEPS):
                rhs = hs[:, :, BS * t : BS * (t + 1)]
                for hc, wave in enumerate(WAVES):
                    for m in wave:
                        for kc in range(2):
                            nc.tensor.matmul(
                                accs[m][:],
                                whhT[:, kc, 128 * m : 128 * (m + 1)],
                                rhs[:, kc, :],
                                start=(kc == 0),
                                stop=(kc == 1),
                            )
                    gt = rsb.tile([128, 4, BS], f32, name=f"gt{t}_{hc}", tag=f"gt{hc}")
                    at = rsb.tile([128, 4, BS], f32, name=f"at{t}_{hc}", tag=f"at{hc}")
                    for j, m in enumerate(wave):
                        nc.vector.tensor_tensor(
                            out=gt[:, j, :],
                            in0=accs[m][:],
                            in1=xg[:, m, BS * t : BS * (t + 1)],
                            op=Alu.add,
                        )
                    # wave order: i, f, g, o at j = 0,1,2,3
                    nc.scalar.activation(at[:, 0, :], gt[:, 0, :], Act.Sigmoid)
                    nc.scalar.activation(at[:, 1, :], gt[:, 1, :], Act.Sigmoid)
                    nc.scalar.activation(at[:, 2, :], gt[:, 2, :], Act.Tanh)
                    nc.scalar.activation(at[:, 3, :], gt[:, 3, :], Act.Sigmoid)
                    t1 = rsb.tile([128, BS], f32, name=f"t1_{t}_{hc}", tag=f"t1{hc}")
                    t2 = rsb.tile([128, BS], f32, name=f"t2_{t}_{hc}", tag=f"t2{hc}")
                    nc.vector.tensor_tensor(t1[:], at[:, 1, :], c_sb[:, hc, :], op=Alu.mult)
                    nc.vector.tensor_tensor(t2[:], at[:, 0, :], at[:, 2, :], op=Alu.mult)
                    nc.vector.tensor_tensor(c_sb[:, hc, :], t1[:], t2[:], op=Alu.add)
                    th = rsb.tile([128, BS], f32, name=f"th{t}_{hc}", tag=f"th{hc}")
                    nc.scalar.activation(th[:], c_sb[:, hc, :], Act.Tanh)
                    nc.vector.tensor_tensor(
                        hs[:, hc, BS * (t + 1) : BS * (t + 2)],
                        at[:, 3, :],
                        th[:],
                        op=Alu.mult,
                    )

        # ---- P4: partial feats + AllReduce ----
        with (
            tc.tile_pool(name="f_psum", bufs=2, space="PSUM") as fpp,
            tc.tile_pool(name="f_sbuf", bufs=1) as fsb,
        ):
            nc.gpsimd.dma_start(pf_d[:], pfz_d[:])
            pf = fsb.tile([128, NT, TAGS], f32)
            for mt in range(NT):
                pfp = fpp.tile([128, TAGS], f32, name=f"pfp{mt}", tag=f"pfp{mt % 4}")
                for kc in range(2):
                    nc.tensor.matmul(
                        pfp[:],
                        hs[:, kc, BS + 128 * mt : BS + 128 * (mt + 1)],
                        woT[:, kc, :],
                        start=(kc == 0),
                        stop=(kc == 1),
                    )
                nc.vector.tensor_tensor(pf[:, mt, :], pfp[:], bo_rep[:], op=Alu.add)
            for g in range(NT):
                nc.gpsimd.indirect_dma_start(
                    out=pf_d[:],
                    out_offset=bass.IndirectOffsetOnAxis(ap=scat_sb[:, g : g + 1], axis=0),
                    in_=pf[:, g, :],
                    in_offset=None,
                )
            nc.gpsimd.collective_compute(
                "AllReduce",
                Alu.add,
                replica_groups=[[0, 1, 2, 3, 4, 5, 6, 7]],
                ins=[pf_d[:]],
                outs=[feats_d[:]],
            )

        # ---- P5: Viterbi ----
        with tc.tile_pool(name="vit", bufs=1) as vp:
            emits = vp.tile([32, S, TAGS], f32)
            for g in range(GRP):
                nc.gpsimd.dma_start(
                    emits[BS * g : BS * (g + 1), :, :],
                    bass.AP(tensor=feats_d, offset=TAGS * TOK * g,
                            ap=[[TAGS, BS], [TAGS * BS, S], [1, TAGS]]),
                )
            fv = vp.tile([32, TAGS], f32)
            nc.gpsimd.dma_start(fv[:], _bcast_ap(fvinit_d[:], 32))
            bp = vp.tile([32, S, TAGS], f32)
            scores = vp.tile([32, TAGS, TAGS], f32)
            mx = vp.tile([32, TAGS], f32)
            cand = vp.tile([32, TAGS, TAGS], f32)
            mask = vp.tile([32, TAGS, TAGS], mybir.dt.uint8)
            for t in range(NSTEPS):
                nc.vector.tensor_tensor(
                    out=scores[:],
                    in0=trans_rep[:].rearrange("p (n q) -> p n q", q=TAGS),
                    in1=_free_bcast(fv[:], TAGS, TAGS),
                    op=Alu.add,
                )
                nc.vector.reduce_sum(mx[:], scores[:], axis=AX.X, op=Alu.max)
                nc.vector.tensor_tensor(
                    out=mask[:], in0=scores[:], in1=_inner_bcast(mx[:], TAGS, TAGS),
                    op=Alu.is_equal,
                )
                nc.vector.select(
                    out=cand[:],
                    mask=mask[:],
                    on_true=iota144[:].rearrange("p (n q) -> p n q", q=TAGS),
                    on_false=big_rep[:].rearrange("p (n q) -> p n q", q=TAGS),
                )
                nc.vector.reduce_sum(bp[:, t, :], cand[:], axis=AX.X, op=Alu.min)
                nc.vector.tensor_tensor(fv[:], mx[:], emits[:, t, :], op=Alu.add)

            term = vp.tile([32, TAGS], f32)
            nc.vector.tensor_tensor(term[:], fv[:], tstop_rep[:], op=Alu.add)
            sc = vp.tile([32, 1], f32)
            nc.vector.reduce_sum(sc[:], term[:], axis=AX.X, op=Alu.max)
            maskt = vp.tile([32, TAGS], mybir.dt.uint8)
            nc.vector.tensor_scalar(
                out=maskt[:], in0=term[:], scalar1=sc[:], scalar2=None, op0=Alu.is_equal
            )
            candt = vp.tile([32, TAGS], f32)
            nc.vector.select(
                out=candt[:], mask=maskt[:], on_true=iota12[:], on_false=big_rep[:, :TAGS]
            )
            best = vp.tile([32, 1], f32)
            nc.vector.reduce_sum(best[:], candt[:], axis=AX.X, op=Alu.min)
            nc.gpsimd.dma_start(score_d[:], sc[:32, :])

            # ---- P6: backtrace ----
            pathf = vp.tile([32, S], f32)
            if NSTEPS < S:
                nc.vector.memset(pathf[:], 0.0)
            cur = vp.tile([32, 1], f32)
            oh = vp.tile([32, TAGS], f32)
            prod = vp.tile([32, TAGS], f32)
            nc.vector.tensor_copy(pathf[:, NSTEPS - 1 : NSTEPS], best[:])
            nc.vector.tensor_scalar(
                out=oh[:], in0=iota12[:], scalar1=best[:], scalar2=None, op0=Alu.is_equal
            )
            for t in range(NSTEPS - 2, -1, -1):
                nc.vector.tensor_tensor(prod[:], bp[:, t + 1, :], oh[:], op=Alu.mult)
                nc.vector.reduce_sum(pathf[:, t : t + 1], prod[:], axis=AX.X, op=Alu.add)
                nc.vector.tensor_scalar(
                    out=oh[:], in0=iota12[:],
                    scalar1=pathf[:, t : t + 1], scalar2=None, op0=Alu.is_equal,
                )
            pathi = vp.tile([32, S], i32)
            nc.vector.tensor_copy(pathi[:], pathf[:])
            nc.gpsimd.dma_start(path_d[:], pathi[:32, :])

        consts_cm.__exit__(None, None, None)

    nc.finalize()
    return nc


def kernel(sentences, emb, wih_f, whh_f, bih_f, bhh_f, wih_b, whh_b,
           bih_b, bhh_b, w_out, b_out, transitions):
    sentences = np.asarray(sentences)
    emb = np.ascontiguousarray(np.asarray(emb, np.float32))

    if "nc" not in _cache:
        _cache["nc"] = _build()
    nc = _cache["nc"]

    iota12 = np.arange(TAGS, dtype=np.float32)
    iota144 = np.tile(iota12, TAGS)
    fv_init = np.full(TAGS, NEG, np.float32)
    fv_init[START_IDX] = 0.0
    trans = np.asarray(transitions, np.float32)

    t_f = np.arange(S)
    t_b = S - 1 - t_f
    pf_zero = np.zeros((GRP * TOK, TAGS), np.float32)
    in_maps = []
    for core in range(NCORES):
        fwd = core < 4
        g = core % 4
        sl = slice(g * BS, (g + 1) * BS)
        sent = np.asarray(sentences[sl], np.int64)  # [8, 256]
        torder = t_f if fwd else t_b
        # token j = t*8 + s
        idx = sent[:, torder].T.reshape(TOK).astype(np.int32)
        jt, js = np.meshgrid(np.arange(S), np.arange(BS), indexing="ij")
        real_t = jt if fwd else (S - 1 - jt)
        scat = (g * TOK + real_t * BS + js).reshape(TOK).astype(np.int32)

        wih = np.asarray(wih_f if fwd else wih_b, np.float32)
        whh = np.asarray(whh_f if fwd else whh_b, np.float32)
        bias = (np.asarray(bih_f if fwd else bih_b, np.float32)
                + np.asarray(bhh_f if fwd else bhh_b, np.float32))
        wo = np.asarray(w_out, np.float32)
        woT = (wo[:, :H] if fwd else wo[:, H:]).T  # [256, 12]
        bo = np.asarray(b_out, np.float32) if fwd else np.zeros(TAGS, np.float32)

        in_maps.append(dict(
            emb_tab=emb,
            sent_idx=idx,
            scat_idx=scat,
            wihT=np.ascontiguousarray(wih.T),
            whhT=np.ascontiguousarray(whh.T),
            bias=np.ascontiguousarray(bias.reshape(8, 128)),
            woT=np.ascontiguousarray(woT),
            bo=bo,
            trans_flat=np.ascontiguousarray(trans.reshape(-1)),
            trans_stop=np.ascontiguousarray(trans[STOP_IDX]),
            fv_init=fv_init,
            iota144=iota144,
            iota12=iota12,
            pf_zero=pf_zero,
        ))

    trace = os.environ.get("KERNEL_TRACE", "0") == "1"
    res = run_bass_kernel_spmd(nc, in_maps, list(range(NCORES)), trace=trace)
    _cache["exec_time_ns"] = res.exec_time_ns
    for k_ in ("emits_dbg", "xt_dbg", "xg_dbg"):
        if k_ in res.results[0]:
            _cache[k_] = res.results[0][k_]
    scores = res.results[0]["score"][:, 0]
    paths = res.results[0]["path"]
    return scores.astype(np.float32), paths.astype(np.int32)


# revision 16
# speedup vs baseline: 1.0244x; 1.0244x over previous
"""BiLSTM-CRF Trainium2 kernel.

Sharding: 8 cores = 2 directions x 4 sequence-groups (8 seqs each).
Core c in 0..3: forward LSTM for seqs [8c, 8c+8); core c+4: backward
LSTM for the same group (host feeds it time-reversed tokens).
Each core: embedding gather -> input projection -> LSTM recurrence
(weight-stationary fp32 matmuls) -> partial CRF emission features;
pair AllReduce combines fwd+bwd partials; every core then runs the
Viterbi scan + backtrace for its group's 8 sequences. Host takes
cores 0-3's outputs.
"""

import os
import numpy as np

import concourse.bacc as bacc
import concourse.bass as bass
import concourse.mybir as mybir
import concourse.tile as tile
from concourse.bass_utils import run_bass_kernel_spmd
from concourse.masks import make_identity

f32 = mybir.dt.float32
i32 = mybir.dt.int32
Alu = mybir.AluOpType
Act = mybir.ActivationFunctionType
AX = mybir.AxisListType

B, S = 32, 256
EMB = 256
H = 256          # per-direction hidden
G4 = 4 * H       # 1024 gates
TAGS = 12
START_IDX, STOP_IDX = 10, 11
NEG = -10000.0
BIGF = 1.0e9

NCORES = 8
GRP = 4          # seq groups
BS = B // GRP    # 8 seqs per core
TOK = BS * S     # 2048 tokens per core
NT = TOK // 128  # 16 token tiles

NSTEPS = int(os.environ.get("KERNEL_NSTEPS", str(S)))

_cache = {}


def _bcast_ap(ap, nrep):
    """Replicate a [1, ...]-partition dram AP across nrep partitions."""
    return bass.AP(tensor=ap.tensor, offset=ap.offset, ap=[[0, nrep]] + list(ap.ap))


def _free_bcast(t_ap, reps, inner):
    """AP reading t_ap ([P, inner]) as [P, reps, inner] with 0-stride reps."""
    return bass.AP(
        tensor=t_ap.tensor,
        offset=t_ap.offset,
        ap=[list(t_ap.ap[0]), [0, reps], [1, inner]],
    )


def _inner_bcast(t_ap, outer, reps):
    """AP reading t_ap ([P, outer]) as [P, outer, reps] with 0-stride reps."""
    return bass.AP(
        tensor=t_ap.tensor,
        offset=t_ap.offset,
        ap=[list(t_ap.ap[0]), [1, outer], [0, reps]],
    )


def _build():
    nc = bacc.Bacc(None, target_bir_lowering=False, enable_partition_id=False)

    emb_d = nc.dram_tensor("emb_tab", [100000, EMB], f32, kind="ExternalInput")
    idx_d = nc.dram_tensor("sent_idx", [TOK], i32, kind="ExternalInput")
    scat_d = nc.dram_tensor("scat_idx", [TOK], i32, kind="ExternalInput")
    wihT_d = nc.dram_tensor("wihT", [EMB, G4], f32, kind="ExternalInput")
    whhT_d = nc.dram_tensor("whhT", [H, G4], f32, kind="ExternalInput")
    bias_d = nc.dram_tensor("bias", [8, 128], f32, kind="ExternalInput")
    woT_d = nc.dram_tensor("woT", [H, TAGS], f32, kind="ExternalInput")
    bo_d = nc.dram_tensor("bo", [TAGS], f32, kind="ExternalInput")
    trans_d = nc.dram_tensor("trans_flat", [TAGS * TAGS], f32, kind="ExternalInput")
    tstop_d = nc.dram_tensor("trans_stop", [TAGS], f32, kind="ExternalInput")
    fvinit_d = nc.dram_tensor("fv_init", [TAGS], f32, kind="ExternalInput")
    iota144_d = nc.dram_tensor("iota144", [TAGS * TAGS], f32, kind="ExternalInput")
    iota12_d = nc.dram_tensor("iota12", [TAGS], f32, kind="ExternalInput")

    pfz_d = nc.dram_tensor("pf_zero", [GRP * TOK, TAGS], f32, kind="ExternalInput")

    score_d = nc.dram_tensor("score", [B, 1], f32, kind="ExternalOutput")
    path_d = nc.dram_tensor("path", [B, S], i32, kind="ExternalOutput")

    pf_d = nc.dram_tensor("pf_buf", [GRP * TOK, TAGS], f32)
    feats_d = nc.dram_tensor("feats_buf", [GRP * TOK, TAGS], f32, addr_space="Shared")

    with tile.TileContext(nc) as tc:
        consts_cm = tc.tile_pool(name="consts", bufs=1)
        consts = consts_cm.__enter__()
        wihT = consts.tile([128, 2, G4], f32)
        nc.gpsimd.dma_start(wihT[:], wihT_d.rearrange("(a b) g -> b a g", b=128))
        whhT = consts.tile([128, 2, G4], f32)
        nc.gpsimd.dma_start(whhT[:], whhT_d.rearrange("(a b) g -> b a g", b=128))
        bias = consts.tile([128, 8], f32)
        nc.gpsimd.dma_start(bias[:], bias_d.rearrange("m p -> p m"))
        woT = consts.tile([128, 2, TAGS], f32)
        nc.gpsimd.dma_start(woT[:], woT_d.rearrange("(a b) n -> b a n", b=128))
        bo_rep = consts.tile([128, TAGS], f32)
        nc.gpsimd.dma_start(bo_rep[:], _bcast_ap(bo_d[:], 128))
        idx_sb = consts.tile([128, NT], i32)
        nc.gpsimd.dma_start(idx_sb[:], idx_d.rearrange("(g p) -> p g", p=128))
        scat_sb = consts.tile([128, NT], i32)
        nc.gpsimd.dma_start(scat_sb[:], scat_d.rearrange("(g p) -> p g", p=128))
        ident = consts.tile([128, 128], f32)
        make_identity(nc, ident[:])

        trans_rep = consts.tile([32, TAGS * TAGS], f32)
        nc.gpsimd.dma_start(trans_rep[:], _bcast_ap(trans_d[:], 32))
        iota144 = consts.tile([32, TAGS * TAGS], f32)
        nc.gpsimd.dma_start(iota144[:], _bcast_ap(iota144_d[:], 32))
        iota12 = consts.tile([32, TAGS], f32)
        nc.gpsimd.dma_start(iota12[:], _bcast_ap(iota12_d[:], 32))
        tstop_rep = consts.tile([32, TAGS], f32)
        nc.gpsimd.dma_start(tstop_rep[:], _bcast_ap(tstop_d[:], 32))
        big_rep = consts.tile([32, TAGS * TAGS], f32)
        nc.vector.memset(big_rep[:], BIGF)

        # big persistent state
        xg = consts.tile([128, 8, TOK], f32)          # pre-biased input projection
        hs = consts.tile([128, 2, BS * (S + 1)], f32)  # h_{-1..S-1}, slot(t)=8(t+1)
        c_sb = consts.tile([128, 2, BS], f32)
        nc.vector.memset(hs[:], 0.0)
        nc.vector.memset(c_sb[:], 0.0)

        # ---- P0/P1: embedding gather + transpose ----
        with (
            tc.tile_pool(name="emb_pool", bufs=3) as embp,
            tc.tile_pool(name="tp_psum", bufs=2, space="PSUM") as tpp,
        ):
            xT = consts.tile([128, 2, TOK], f32)
            for g in range(NT):
                xrow = embp.tile([128, EMB], f32, name=f"xrow{g}", tag="xrow")
                nc.gpsimd.indirect_dma_start(
                    out=xrow[:],
                    out_offset=None,
                    in_=emb_d[:],
                    in_offset=bass.IndirectOffsetOnAxis(ap=idx_sb[:, g : g + 1], axis=0),
                )
                for kc in range(2):
                    pt = tpp.tile([128, 128], f32, name=f"pt{g}_{kc}", tag=f"pt{kc}")
                    nc.tensor.transpose(
                        out=pt[:], in_=xrow[:, 128 * kc : 128 * (kc + 1)], identity=ident[:]
                    )
                    nc.vector.tensor_copy(xT[:, kc, 128 * g : 128 * (g + 1)], pt[:])

        # ---- P2: input projection xg = wih @ x + bias ----
        with tc.tile_pool(name="xg_psum", bufs=2, space="PSUM") as xgp:
            for m in range(8):
                for nb in range(TOK // 512):
                    px = xgp.tile([128, 512], f32, name=f"px{m}_{nb}", tag=f"px{nb % 2}")
                    for kc in range(2):
                        nc.tensor.matmul(
                            px[:],
                            wihT[:, kc, 128 * m : 128 * (m + 1)],
                            xT[:, kc, 512 * nb : 512 * (nb + 1)],
                            start=(kc == 0),
                            stop=(kc == 1),
                        )
                    nc.vector.tensor_scalar(
                        out=xg[:, m, 512 * nb : 512 * (nb + 1)],
                        in0=px[:],
                        scalar1=bias[:, m : m + 1],
                        scalar2=None,
                        op0=Alu.add,
                    )

        # ---- P3: LSTM recurrence ----
        with (
            tc.tile_pool(name="rec_psum", bufs=1, space="PSUM") as rpp,
            tc.tile_pool(name="rec_sbuf", bufs=3) as rsb,
        ):
            accs = [rpp.tile([128, BS], f32, name=f"racc{m}", tag=f"racc{m}") for m in range(8)]
            WAVES = ((0, 2, 4, 6), (1, 3, 5, 7))
            for t in range(NSTEPS):
                rhs = hs[:, :, BS * t : BS * (t + 1)]
                # kc=0 pass needs only h-chunk 0 (ready early); kc=1 pass
                # ordered evens-first so the h0 tail can start while the
                # odd gate-chunks are still streaming.
                for m in range(8):
                    nc.tensor.matmul(
                        accs[m][:], whhT[:, 0, 128 * m : 128 * (m + 1)],
                        rhs[:, 0, :], start=True, stop=False,
                    )
                for m in (0, 2, 4, 6, 1, 3, 5, 7):
                    nc.tensor.matmul(
                        accs[m][:], whhT[:, 1, 128 * m : 128 * (m + 1)],
                        rhs[:, 1, :], start=False, stop=True,
                    )
                for hc, wave in enumerate(WAVES):
                    gt = rsb.tile([128, 4, BS], f32, name=f"gt{t}_{hc}", tag=f"gt{hc}")
                    at = rsb.tile([128, 4, BS], f32, name=f"at{t}_{hc}", tag=f"at{hc}")
                    for j, m in enumerate(wave):
                        nc.vector.tensor_tensor(
                            out=gt[:, j, :],
                            in0=accs[m][:],
                            in1=xg[:, m, BS * t : BS * (t + 1)],
                            op=Alu.add,
                        )
                    # wave order: i, f, g, o at j = 0,1,2,3
                    nc.scalar.activation(at[:, 0, :], gt[:, 0, :], Act.Sigmoid)
                    nc.scalar.activation(at[:, 1, :], gt[:, 1, :], Act.Sigmoid)
                    nc.scalar.activation(at[:, 2, :], gt[:, 2, :], Act.Tanh)
                    nc.scalar.activation(at[:, 3, :], gt[:, 3, :], Act.Sigmoid)
                    t1 = rsb.tile([128, BS], f32, name=f"t1_{t}_{hc}", tag=f"t1{hc}")
                    t2 = rsb.tile([128, BS], f32, name=f"t2_{t}_{hc}", tag=f"t2{hc}")
                    nc.vector.tensor_tensor(t1[:], at[:, 1, :], c_sb[:, hc, :], op=Alu.mult)
                    nc.vector.tensor_tensor(t2[:], at[:, 0, :], at[:, 2, :], op=Alu.mult)
                    nc.vector.tensor_tensor(c_sb[:, hc, :], t1[:], t2[:], op=Alu.add)
                    th = rsb.tile([128, BS], f32, name=f"th{t}_{hc}", tag=f"th{hc}")
                    nc.scalar.activation(th[:], c_sb[:, hc, :], Act.Tanh)
                    nc.vector.tensor_tensor(
                        hs[:, hc, BS * (t + 1) : BS * (t + 2)],
                        at[:, 3, :],
                        th[:],
                        op=Alu.mult,
                    )

        # ---- P4: partial feats + AllReduce ----
        with (
            tc.tile_pool(name="f_psum", bufs=2, space="PSUM") as fpp,
            tc.tile_pool(name="f_sbuf", bufs=1) as fsb,
        ):
            nc.gpsimd.dma_start(pf_d[:], pfz_d[:])
            pf = fsb.tile([128, NT, TAGS], f32)
            for mt in range(NT):
                pfp = fpp.tile([128, TAGS], f32, name=f"pfp{mt}", tag=f"pfp{mt % 4}")
                for kc in range(2):
                    nc.tensor.matmul(
                        pfp[:],
                        hs[:, kc, BS + 128 * mt : BS + 128 * (mt + 1)],
                        woT[:, kc, :],
                        start=(kc == 0),
                        stop=(kc == 1),
                    )
                nc.vector.tensor_tensor(pf[:, mt, :], pfp[:], bo_rep[:], op=Alu.add)
            for g in range(NT):
                nc.gpsimd.indirect_dma_start(
                    out=pf_d[:],
                    out_offset=bass.IndirectOffsetOnAxis(ap=scat_sb[:, g : g + 1], axis=0),
                    in_=pf[:, g, :],
                    in_offset=None,
                )
            nc.gpsimd.collective_compute(
                "AllReduce",
                Alu.add,
                replica_groups=[[0, 1, 2, 3, 4, 5, 6, 7]],
                ins=[pf_d[:]],
                outs=[feats_d[:]],
            )

        # ---- P5: Viterbi ----
        with tc.tile_pool(name="vit", bufs=1) as vp:
            emits = vp.tile([32, S, TAGS], f32)
            for g in range(GRP):
                nc.gpsimd.dma_start(
                    emits[BS * g : BS * (g + 1), :, :],
                    bass.AP(tensor=feats_d, offset=TAGS * TOK * g,
                            ap=[[TAGS, BS], [TAGS * BS, S], [1, TAGS]]),
                )
            fvh = vp.tile([32, S + 1, TAGS], f32)
            nc.gpsimd.dma_start(fvh[:, 0, :], _bcast_ap(fvinit_d[:], 32))
            scores = vp.tile([32, TAGS, TAGS], f32)
            mx = vp.tile([32, TAGS], f32)
            # forward scan: only max + emit on the serial chain
            for t in range(NSTEPS):
                nc.vector.tensor_tensor(
                    out=scores[:],
                    in0=trans_rep[:].rearrange("p (n q) -> p n q", q=TAGS),
                    in1=_free_bcast(fvh[:, t, :], TAGS, TAGS),
                    op=Alu.add,
                )
                nc.vector.reduce_sum(mx[:], scores[:], axis=AX.X, op=Alu.max)
                nc.vector.tensor_tensor(fvh[:, t + 1, :], mx[:], emits[:, t, :], op=Alu.add)
            fv = fvh[:, NSTEPS, :]

            # batched backpointer sweep over 128 partitions = (tchunk, seq)
            TC, TCL = 4, S // 4  # 4 chunks of 64 steps
            fvr = vp.tile([128, TCL, TAGS], f32)
            for tcb in range(TC):
                nc.gpsimd.dma_start(
                    fvr[32 * tcb : 32 * (tcb + 1), :, :],
                    fvh[:, TCL * tcb : TCL * tcb + TCL, :],
                )
            trans2 = vp.tile([128, TAGS * TAGS], f32)
            nc.gpsimd.dma_start(trans2[:], _bcast_ap(trans_d[:], 128))
            iota2 = vp.tile([128, TAGS], f32)
            nc.gpsimd.dma_start(iota2[:], _bcast_ap(iota12_d[:], 128))
            big2 = vp.tile([128, TAGS], f32)
            nc.vector.memset(big2[:], BIGF)
            bp2 = vp.tile([128, TCL, TAGS], f32)
            HALF = TCL // 2  # 32 steps per batch
            sc2 = vp.tile([128, HALF, TAGS, TAGS], f32)
            mx2 = vp.tile([128, HALF, TAGS], f32)
            msk2 = vp.tile([128, HALF, TAGS, TAGS], mybir.dt.uint8)
            cnd2 = vp.tile([128, HALF, TAGS, TAGS], f32)
            for hb in range(2):
                off = hb * HALF
                nc.vector.tensor_tensor(
                    out=sc2[:],
                    in0=bass.AP(tensor=trans2[:].tensor, offset=trans2[:].offset,
                                ap=[list(trans2[:].ap[0]), [0, HALF], [12, TAGS], [1, TAGS]]),
                    in1=bass.AP(tensor=fvr[:].tensor, offset=fvr[:].offset + TAGS * off,
                                ap=[list(fvr[:].ap[0]), [TAGS, HALF], [0, TAGS], [1, TAGS]]),
                    op=Alu.add,
                )
                nc.vector.reduce_sum(mx2[:], sc2[:], axis=AX.X, op=Alu.max)
                nc.vector.tensor_tensor(
                    out=msk2[:], in0=sc2[:],
                    in1=bass.AP(tensor=mx2[:].tensor, offset=mx2[:].offset,
                                ap=[list(mx2[:].ap[0]), [TAGS, HALF], [1, TAGS], [0, TAGS]]),
                    op=Alu.is_equal,
                )
                nc.vector.select(
                    out=cnd2[:], mask=msk2[:],
                    on_true=bass.AP(tensor=iota2[:].tensor, offset=iota2[:].offset,
                                    ap=[list(iota2[:].ap[0]), [0, HALF], [0, TAGS], [1, TAGS]]),
                    on_false=bass.AP(tensor=big2[:].tensor, offset=big2[:].offset,
                                     ap=[list(big2[:].ap[0]), [0, HALF], [0, TAGS], [1, TAGS]]),
                )
                nc.vector.reduce_sum(bp2[:, off : off + HALF, :], cnd2[:], axis=AX.X, op=Alu.min)
            bp = vp.tile([32, S, TAGS], f32)
            for tcb in range(TC):
                nc.gpsimd.dma_start(
                    bp[:, TCL * tcb : TCL * tcb + TCL, :],
                    bp2[32 * tcb : 32 * (tcb + 1), :, :],
                )

            term = vp.tile([32, TAGS], f32)
            nc.vector.tensor_tensor(term[:], fv, tstop_rep[:], op=Alu.add)
            sc = vp.tile([32, 1], f32)
            nc.vector.reduce_sum(sc[:], term[:], axis=AX.X, op=Alu.max)
            maskt = vp.tile([32, TAGS], mybir.dt.uint8)
            nc.vector.tensor_scalar(
                out=maskt[:], in0=term[:], scalar1=sc[:], scalar2=None, op0=Alu.is_equal
            )
            candt = vp.tile([32, TAGS], f32)
            nc.vector.select(
                out=candt[:], mask=maskt[:], on_true=iota12[:], on_false=big_rep[:, :TAGS]
            )
            best = vp.tile([32, 1], f32)
            nc.vector.reduce_sum(best[:], candt[:], axis=AX.X, op=Alu.min)
            nc.gpsimd.dma_start(score_d[:], sc[:32, :])

            # ---- P6: backtrace ----
            pathf = vp.tile([32, S], f32)
            if NSTEPS < S:
                nc.vector.memset(pathf[:], 0.0)
            cur = vp.tile([32, 1], f32)
            oh = vp.tile([32, TAGS], f32)
            prod = vp.tile([32, TAGS], f32)
            nc.vector.tensor_copy(pathf[:, NSTEPS - 1 : NSTEPS], best[:])
            nc.vector.tensor_scalar(
                out=oh[:], in0=iota12[:], scalar1=best[:], scalar2=None, op0=Alu.is_equal
            )
            for t in range(NSTEPS - 2, -1, -1):
                nc.vector.tensor_tensor(prod[:], bp[:, t + 1, :], oh[:], op=Alu.mult)
                nc.vector.reduce_sum(pathf[:, t : t + 1], prod[:], axis=AX.X, op=Alu.add)
                nc.vector.tensor_scalar(
                    out=oh[:], in0=iota12[:],
                    scalar1=pathf[:, t : t + 1], scalar2=None, op0=Alu.is_equal,
                )
            pathi = vp.tile([32, S], i32)
            nc.vector.tensor_copy(pathi[:], pathf[:])
            nc.gpsimd.dma_start(path_d[:], pathi[:32, :])

        consts_cm.__exit__(None, None, None)

    nc.finalize()
    return nc


def kernel(sentences, emb, wih_f, whh_f, bih_f, bhh_f, wih_b, whh_b,
           bih_b, bhh_b, w_out, b_out, transitions):
    sentences = np.asarray(sentences)
    emb = np.ascontiguousarray(np.asarray(emb, np.float32))

    if "nc" not in _cache:
        _cache["nc"] = _build()
    nc = _cache["nc"]

    iota12 = np.arange(TAGS, dtype=np.float32)
    iota144 = np.tile(iota12, TAGS)
    fv_init = np.full(TAGS, NEG, np.float32)
    fv_init[START_IDX] = 0.0
    trans = np.asarray(transitions, np.float32)

    t_f = np.arange(S)
    t_b = S - 1 - t_f
    pf_zero = np.zeros((GRP * TOK, TAGS), np.float32)
    in_maps = []
    for core in range(NCORES):
        fwd = core < 4
        g = core % 4
        sl = slice(g * BS, (g + 1) * BS)
        sent = np.asarray(sentences[sl], np.int64)  # [8, 256]
        torder = t_f if fwd else t_b
        # token j = t*8 + s
        idx = sent[:, torder].T.reshape(TOK).astype(np.int32)
        jt, js = np.meshgrid(np.arange(S), np.arange(BS), indexing="ij")
        real_t = jt if fwd else (S - 1 - jt)
        scat = (g * TOK + real_t * BS + js).reshape(TOK).astype(np.int32)

        wih = np.asarray(wih_f if fwd else wih_b, np.float32)
        whh = np.asarray(whh_f if fwd else whh_b, np.float32)
        bias = (np.asarray(bih_f if fwd else bih_b, np.float32)
                + np.asarray(bhh_f if fwd else bhh_b, np.float32))
        wo = np.asarray(w_out, np.float32)
        woT = (wo[:, :H] if fwd else wo[:, H:]).T  # [256, 12]
        bo = np.asarray(b_out, np.float32) if fwd else np.zeros(TAGS, np.float32)

        in_maps.append(dict(
            emb_tab=emb,
            sent_idx=idx,
            scat_idx=scat,
            wihT=np.ascontiguousarray(wih.T),
            whhT=np.ascontiguousarray(whh.T),
            bias=np.ascontiguousarray(bias.reshape(8, 128)),
            woT=np.ascontiguousarray(woT),
            bo=bo,
            trans_flat=np.ascontiguousarray(trans.reshape(-1)),
            trans_stop=np.ascontiguousarray(trans[STOP_IDX]),
            fv_init=fv_init,
            iota144=iota144,
            iota12=iota12,
            pf_zero=pf_zero,
        ))

    trace = os.environ.get("KERNEL_TRACE", "0") == "1"
    res = run_bass_kernel_spmd(nc, in_maps, list(range(NCORES)), trace=trace)
    _cache["exec_time_ns"] = res.exec_time_ns
    for k_ in ("emits_dbg", "xt_dbg", "xg_dbg"):
        if k_ in res.results[0]:
            _cache[k_] = res.results[0][k_]
    scores = res.results[0]["score"][:, 0]
    paths = res.results[0]["path"]
    return scores.astype(np.float32), paths.astype(np.int32)
